# revision 38
# baseline (speedup 1.0000x reference)
"""Multi-head self-attention with RPE bias — Bass/Tile kernel for 8 TRN2 NeuronCores.

Sharding: core c handles batch c//4 and heads [4*(c%4), 4*(c%4)+4).
Each core returns a row-split partial of the output projection; the host
sums the 4 per-batch partials and adds b_out (+ bv@w_out, exactly).

Design (v4):
- Toeplitz RPE bias split host-side into a rank-63 smooth part (+1 exact
  b_qkv-correction row) riding the otherwise-idle 64 PE contraction rows of
  the scores matmul, plus an exact narrow band (|d|<=127) applied in exp
  space on the ~5/16 near-diagonal tiles.
- Q/K biases handled exactly: q·bk is constant per query and cancels in
  softmax; bq·k is linear in x and folded into factor row 63 on host.
- V projection emitted directly in [token, head-dim] orientation
  (stationary = xt block, moving = wv columns): no PE transposes, one fused
  PSUM->SBUF cast per 128-token block; ones column gives softmax
  denominators for free in the PV matmul.
- Denominator reciprocals via the custom DVE exponent-flip Newton op
  (reciprocal_approx_fast; input must sit at partition 0 — the op misreads
  partition-offset APs), broadcast across partitions by a 1-row f16 matmul.
- Schedule: V wave -> QK wave A (heads 0,1) interleaved with attention unit
  0 per 4-jb group -> wave B (heads 2,3) interleaved with unit 1 -> units
  2..7 with previous-unit normalizes popped at unit boundaries (their bc
  matmuls fill the PE while ACT drains the exp backlog) and out-projection
  blocks popped at late jb slots once their o4 region is normalized.
- DMA: xt streamed in per-half-T tiles over two HWDGE queues (dep tracking
  is tile-granular); factor/band/wout DMAs issue-gated behind the V wave so
  the SDMA round-robin keeps early HBM bandwidth on the projection-critical
  stream; exp ACT table preloaded via a dummy activation during the ramp.
- PSUM budget (8 banks): st 2x[128,1024] (4) + PV accumulators 2x[65,512]
  (2) + outproj/bc pool (2); transient V/QK-wave pools live in the slack
  before the op pool is created.
"""
import numpy as np
import ml_dtypes
from contextlib import ExitStack

import concourse.bass as bass
import concourse.tile as tile
from concourse import bacc, mybir
from concourse.bass import ts
from concourse.bass_utils import run_bass_kernel_spmd

N_CORES = 8
B = 2
T = 2048
C = 1024
H = 16
DH = 64
HEADS_PER_CORE = 4
RANK = 64            # factor rows (63 SVD + 1 bias-correction row)
RANK_SVD = 63
KW = 127             # exact band half-width
WBB = 1408           # band buffer width
BAND_BASE = 639      # slice start = (c0 - 128*jb) + BAND_BASE

F32 = mybir.dt.float32
F32R = mybir.dt.float32r
F16 = mybir.dt.float16
BF16 = mybir.dt.bfloat16

NJB = T // 128
NTB = T // 128
NKC = C // 128


def build_program():
    nc = bacc.Bacc("TRN2", target_bir_lowering=False, debug=False, num_devices=N_CORES)

    xt_d = nc.dram_tensor("xt", [NKC, 128, T], BF16, kind="ExternalInput").ap()
    wqk_d = nc.dram_tensor("wqk", [128, HEADS_PER_CORE * NKC * 128], BF16, kind="ExternalInput").ap()
    wv_d = nc.dram_tensor("wv", [128, NKC * 4 * DH], BF16, kind="ExternalInput").ap()
    wout_d = nc.dram_tensor("wout", [128, 2 * C], F16, kind="ExternalInput").ap()
    gmov_d = nc.dram_tensor("gmov", [RANK, HEADS_PER_CORE, T], F16, kind="ExternalInput").ap()
    fstat_d = nc.dram_tensor("fstat", [RANK, HEADS_PER_CORE, T], F16, kind="ExternalInput").ap()
    wbb_d = nc.dram_tensor("wbb", [128, HEADS_PER_CORE, WBB], F16, kind="ExternalInput").ap()
    onesr_d = nc.dram_tensor("onesr", [1, DH], F16, kind="ExternalInput").ap()
    out_d = nc.dram_tensor("out", [T, C], F16, kind="ExternalOutput").ap()
    scr_d = nc.dram_tensor("scr", [1, 8], F16, kind="ExternalOutput").ap()

    with tile.TileContext(nc) as tc, ExitStack() as ctx:
        # ---------- persistent pools ----------
        const_pool = ctx.enter_context(tc.tile_pool(name="const", bufs=1))
        qg_pool = ctx.enter_context(tc.tile_pool(name="qg", bufs=1))
        v_pool = ctx.enter_context(tc.tile_pool(name="v", bufs=1))
        wbb_pool = ctx.enter_context(tc.tile_pool(name="wbb", bufs=1))
        o4_pool = ctx.enter_context(tc.tile_pool(name="o4", bufs=1))

        onesr_sb = const_pool.tile([1, DH], F16, tag="onesr")
        nc.sync.dma_start(onesr_sb[:], onesr_d[:])
        onesr32_sb = const_pool.tile([1, DH], F32, tag="onesr32")
        nc.gpsimd.memset(onesr32_sb[:], 1.0)

        # preload the exp table set while DMAs ramp (saves the ~2.7us
        # ACT_TABLE_LOAD stall at the first real exp)
        dummy = const_pool.tile([1, 8], F32, tag="dummy")
        nc.gpsimd.memset(dummy[:], 0.0)
        nc.scalar.activation(dummy[:], dummy[:], mybir.ActivationFunctionType.Exp)

        # per head: rows 0:64 q/k (dh-major), rows 64:128 bias factors
        qg4 = qg_pool.tile([128, HEADS_PER_CORE, T], F16, tag="qg4", name="qg4")
        kf4 = qg_pool.tile([128, HEADS_PER_CORE, T], F16, tag="kf4", name="kf4")
        # V in [token-within-block, (jb, head, dh+ones)] layout
        v4_sb = v_pool.tile([128, NJB, HEADS_PER_CORE, DH + 1], F16, tag="v4", name="v4")
        wbb4 = wbb_pool.tile([128, HEADS_PER_CORE, WBB], F16, tag="wbb4", name="wbb4")
        o4_sb = [o4_pool.tile([128, T], F16, tag=f"o4_{a}", name=f"o4_{a}") for a in range(2)]

        nc.gpsimd.memset(v4_sb[:, :, :, DH : DH + 1], 1.0)

        # ---------- attention pools (created first: they outlive the
        # transient projection pools; tags allocate lazily on first tile) ----
        st_pool = ctx.enter_context(tc.tile_pool(name="st_ps", bufs=2, space="PSUM"))
        ot_pool = ctx.enter_context(tc.tile_pool(name="ot_ps", bufs=2, space="PSUM"))
        pt_pool = ctx.enter_context(tc.tile_pool(name="pt", bufs=6))
        ptm_pool = ctx.enter_context(tc.tile_pool(name="ptm", bufs=6))
        otu_pool = ctx.enter_context(tc.tile_pool(name="otu", bufs=4))
        nrm_pool = ctx.enter_context(tc.tile_pool(name="nrm", bufs=4))
        out_pool = ctx.enter_context(tc.tile_pool(name="out_sb", bufs=4))

        # ---------- DMAs ----------
        xt_pool = ctx.enter_context(tc.tile_pool(name="xt", bufs=8))
        w_pool = ctx.enter_context(tc.tile_pool(name="wproj", bufs=1))

        wv_sb = w_pool.tile([128, NKC, 4 * DH], BF16, tag="wv")
        nc.gpsimd.dma_start(wv_sb[:], wv_d[:])
        wqk_sb = w_pool.tile([128, HEADS_PER_CORE * NKC * 128], BF16, tag="wqk")
        nc.gpsimd.dma_start(wqk_sb[:, 0 : 2 * NKC * 128], wqk_d[:, 0 : 2 * NKC * 128])

        # xt in half-T chunks over two queues; each half is its OWN tile
        # because the dependency tracker is tile-granular -- a single tile
        # written by two DMAs would stall first readers on the second DMA
        xt_sb = [[xt_pool.tile([128, 1024], BF16, tag=f"xt{half}", name=f"xt{kc}_{half}")
                  for half in range(2)] for kc in range(NKC)]
        for half in range(2):
            for kc in range(NKC):
                eng = nc.sync if kc % 2 == 0 else nc.scalar
                eng.dma_start(xt_sb[kc][half][:], xt_d[kc][:, ts(half, 1024)])

        def xt_ap(kc, col, width):
            half, off = col // 1024, col % 1024
            assert off + width <= 1024
            return xt_sb[kc][half][:, off : off + width]

        wout_sb = w_pool.tile([128, 2 * C], F16, tag="wout")

        # ---------- V wave: out[token, head-dim], ones column at DH ----------
        with ExitStack() as pv:
            v_ps_pool = pv.enter_context(tc.tile_pool(name="v_ps", bufs=2, space="PSUM"))
            for tb in range(NTB):
                psv = v_ps_pool.tile([128, 4 * DH], F32, tag="vps", name=f"vps{tb}")
                for kc in range(NKC):
                    nc.tensor.matmul(
                        psv[:], xt_ap(kc, tb * 128, 128), wv_sb[:, kc, :],
                        start=(kc == 0), stop=(kc == NKC - 1),
                    )
                nc.vector.tensor_copy(v4_sb[:, tb, :, 0:DH], psv[:])

        # factor rows / band / wout deferred until the V wave completes: the
        # SDMA pool round-robins between queues at packet granularity, so a
        # late ISSUE keeps the first ~5.5MB of HBM bandwidth on the
        # projection-critical xt/wqk/wv stream. The throwaway scr DMA's
        # data-dependency on the last V copy stalls the sync queue until
        # ~17us; everything still lands well before u0 needs it (~40us).
        nc.sync.dma_start(scr_d[:], v4_sb[0:1, 7, 3, 0:8])
        nc.sync.dma_start(qg4[DH:128, 0:2, :], gmov_d[:, 0:2, :])
        nc.sync.dma_start(kf4[DH:128, 0:2, :], fstat_d[:, 0:2, :])
        nc.sync.dma_start(wbb4[:, 0:2, :], wbb_d[:, 0:2, :])
        # wave-B half of the QKV weights (h2,h3) is first consumed ~45us in
        nc.sync.dma_start(wqk_sb[:, 2 * NKC * 128 :], wqk_d[:, 2 * NKC * 128 :])
        nc.sync.dma_start(qg4[DH:128, 2:4, :], gmov_d[:, 2:4, :])
        nc.sync.dma_start(kf4[DH:128, 2:4, :], fstat_d[:, 2:4, :])
        nc.sync.dma_start(wbb4[:, 2:4, :], wbb_d[:, 2:4, :])
        nc.sync.dma_start(wout_sb[:], wout_d[:])

        # ---------- QK wave helper ----------
        def qk_copy(eng, dst4, apair, tch, ps):
            if eng is nc.scalar:
                nc.scalar.copy(dst4[0:DH, 2 * apair, ts(tch, 512)], ps[0:DH, :])
                nc.scalar.copy(dst4[0:DH, 2 * apair + 1, ts(tch, 512)], ps[DH:128, :])
            else:
                nc.vector.tensor_copy(dst4[0:DH, 2 * apair, ts(tch, 512)], ps[0:DH, :])
                nc.vector.tensor_copy(dst4[0:DH, 2 * apair + 1, ts(tch, 512)], ps[DH:128, :])

        def qk_wave_tch(pool, apair, tch, engQ, engK):
            # all-Q-then-all-K per tch: the Q copies overlap the K matmuls,
            # so only the K copies sit on the critical psum-recycle path
            psQ = pool.tile([128, 512], F32, tag="pQ", name=f"pQ{apair}_{tch}")
            psK = pool.tile([128, 512], F32, tag="pK", name=f"pK{apair}_{tch}")
            for kc in range(NKC):
                nc.tensor.matmul(
                    psK[:], wqk_sb[:, ts((2 * apair + 1) * NKC + kc, 128)],
                    xt_ap(kc, tch * 512, 512),
                    start=(kc == 0), stop=(kc == NKC - 1),
                )
            qk_copy(engK, kf4, apair, tch, psK)
            for kc in range(NKC):
                nc.tensor.matmul(
                    psQ[:], wqk_sb[:, ts((2 * apair) * NKC + kc, 128)],
                    xt_ap(kc, tch * 512, 512),
                    start=(kc == 0), stop=(kc == NKC - 1),
                )
            qk_copy(engQ, qg4, apair, tch, psQ)

        # ---------- attention units interleaved with QK waves ----------
        norms = {}      # ui -> [cb, cb]

        def make_outproj(tb, split_copy=False):
            def outproj(tb=tb):
                # per-chk tiles + DMAs: dep tracking is tile-granular, so one
                # [128,C] tile would hold the first half's DMA hostage to the
                # second half's copy
                for chk in range(2):
                    ot = out_pool.tile([128, 512], F16, tag="out", name=f"o{tb}_{chk}")
                    op = op_pool.tile([128, 512], F32, tag="op", name=f"op{tb}_{chk}")
                    for a in range(2):
                        nc.tensor.matmul(
                            op[:],
                            o4_sb[a][:, ts(tb, 128)],
                            wout_sb[:, a * C + chk * 512 : a * C + (chk + 1) * 512],
                            start=(a == 0), stop=(a == 1),
                        )
                    if split_copy and chk == 0:
                        nc.scalar.copy(ot[:], op[:])
                    else:
                        nc.vector.tensor_copy(ot[:], op[:])
                    (nc.sync if tb % 2 == 0 else nc.gpsimd).dma_start(
                        out_d[ts(tb, 128), chk * 512 : (chk + 1) * 512], ot[:])
            return outproj

        # unit order: (a, ih, chk) pairs per column range so outproj frees early
        unit_list = [(0, 0, 0), (1, 0, 0), (0, 0, 1), (1, 0, 1),
                     (0, 1, 0), (1, 1, 0), (0, 1, 1), (1, 1, 1)]

        def start_unit(ui):
            a, ih, chk = unit_list[ui]
            return {
                "ui": ui, "a": a, "c0": ih * 1024 + chk * 512,
                "hpair": (2 * a, 2 * a + 1),
                "ot_ps": {hh: ot_pool.tile([DH + 1, 512], F32, tag="ot", name=f"ot{ui}_{hh}")
                          for hh in range(2)},
                "prev": None,
            }

        def emit_pv(stt, jb, prev):
            for hh in range(2):
                p = prev[hh]
                mv = p[0][:, ts(p[1], 512)] if isinstance(p, tuple) else p[:]
                nc.tensor.matmul(
                    stt["ot_ps"][hh][:], v4_sb[:, jb, stt["hpair"][hh], :], mv,
                    start=(jb == 0), stop=(jb == NJB - 1),
                )

        def unit_jbs(stt, jb_lo, jb_hi, pops_at={}):
            ui, c0, hpair = stt["ui"], stt["c0"], stt["hpair"]
            for jb in range(jb_lo, jb_hi):
                if jb in pops_at:
                    pops_at[jb]()
                st = st_pool.tile([128, 1024], F32, tag="st", name=f"st{ui}_{jb}")
                for hh in range(2):
                    nc.tensor.matmul(
                        st[:, ts(hh, 512)],
                        kf4[:, hpair[hh], ts(jb, 128)],
                        qg4[:, hpair[hh], c0 : c0 + 512],
                        start=True, stop=True,
                    )
                if stt["prev"] is not None:
                    emit_pv(stt, jb - 1, stt["prev"])
                pte = pt_pool.tile([128, 1024], F16, tag="pt", name=f"pt{ui}_{jb}")
                nc.scalar.activation(pte[:], st[:], mybir.ActivationFunctionType.Exp)
                cur = {}
                o = c0 - 128 * jb
                for hh in range(2):
                    if -640 < o < 255:
                        ptm = ptm_pool.tile([128, 512], F16, tag="ptm", name=f"ptm{ui}_{jb}_{hh}")
                        nc.vector.tensor_mul(
                            ptm[:], pte[:, ts(hh, 512)],
                            wbb4[:, hpair[hh], o + BAND_BASE : o + BAND_BASE + 512],
                        )
                        cur[hh] = ptm
                    else:
                        cur[hh] = (pte, hh)
                stt["prev"] = cur

        def finish_unit(stt):
            ui, a, c0 = stt["ui"], stt["a"], stt["c0"]
            emit_pv(stt, NJB - 1, stt["prev"])
            tiles = {}
            for hh in range(2):
                tiles[hh] = (
                    otu_pool.tile([DH, 512], F32, tag="otu", name=f"otu{ui}_{hh}"),
                    nrm_pool.tile([1, 512], F32, tag="dent", name=f"dent{ui}_{hh}"),
                )
            if ui == 7:
                # flush: no successor waits on the ot banks, so stage the den
                # rows first -- the serial recip chain is the tail's critical
                # path. (den row to partition 0: the custom DVE reciprocal
                # misreads partition-offset inputs.)
                for hh in range(2):
                    nc.vector.tensor_copy(tiles[hh][1][:], stt["ot_ps"][hh][DH : DH + 1, :])
                for hh in range(2):
                    nc.vector.tensor_copy(tiles[hh][0][:], stt["ot_ps"][hh][0:DH, :])
            ncb = []
            for hh in range(2):
                otu, dent = tiles[hh]
                if ui != 7:
                    nc.vector.tensor_copy(otu[:], stt["ot_ps"][hh][0:DH, :])
                    # den row: partition-64 -> partition-0 (custom DVE op
                    # below misreads partition-offset inputs)
                    nc.vector.tensor_copy(dent[:], stt["ot_ps"][hh][DH : DH + 1, :])

                def normalize(a=a, hh=hh, c0=c0, otu=otu, dent=dent, ui=ui):
                    rec = nrm_pool.tile([1, 512], F32, tag="rec", name=f"rec{ui}_{hh}")
                    nc.vector.reciprocal_approx_fast(rec[:], dent[:])
                    if ui == 7:
                        # flush: PE is idle, the DVE chain is critical ->
                        # skip the f16 cast, broadcast with an f32 matmul
                        rmv = rec
                    else:
                        rmv = nrm_pool.tile([1, 512], F16, tag="rec16", name=f"rec16_{ui}_{hh}")
                        nc.vector.tensor_copy(rmv[:], rec[:])
                    bc = op_pool.tile([DH, 512], F32, tag="op", name=f"bc{ui}_{hh}")
                    nc.tensor.matmul(
                        bc[:], onesr32_sb[:] if ui == 7 else onesr_sb[:], rmv[:],
                        start=True, stop=True,
                    )
                    nc.vector.tensor_mul(
                        o4_sb[a][hh * DH : (hh + 1) * DH, c0 : c0 + 512],
                        otu[0:DH, :], bc[:],
                    )
                ncb.append(normalize)
            norms[ui] = ncb

        # --- u0 interleaved with wave A (heads 0,1; psum: projA 2 + st 4 + ot 2) ---
        with ExitStack() as pa:
            projA = pa.enter_context(tc.tile_pool(name="projA", bufs=1, space="PSUM"))
            qk_wave_tch(projA, 0, 0, nc.scalar, nc.vector)
            qk_wave_tch(projA, 0, 1, nc.scalar, nc.vector)
            s0 = start_unit(0)
            unit_jbs(s0, 0, 4)
            qk_wave_tch(projA, 0, 2, nc.scalar, nc.vector)
            unit_jbs(s0, 4, 12)
            qk_wave_tch(projA, 0, 3, nc.scalar, nc.vector)
            unit_jbs(s0, 12, 16)
            finish_unit(s0)

        # --- u1 interleaved with wave B (heads 2,3) ---
        with ExitStack() as pb:
            projB = pb.enter_context(tc.tile_pool(name="projB", bufs=1, space="PSUM"))
            qk_wave_tch(projB, 1, 0, nc.vector, nc.vector)
            qk_wave_tch(projB, 1, 1, nc.vector, nc.vector)
            s1 = start_unit(1)
            unit_jbs(s1, 0, 4)
            qk_wave_tch(projB, 1, 2, nc.vector, nc.vector)
            unit_jbs(s1, 4, 12)
            qk_wave_tch(projB, 1, 3, nc.vector, nc.vector)

        # create op_pool here: its alloc boundary (which waits on projB's
        # release, i.e. the wave-B copy tail) hides under u1's last jb block
        op_pool = ctx.enter_context(tc.tile_pool(name="op_ps", bufs=2, space="PSUM"))
        unit_jbs(s1, 12, 16)
        finish_unit(s1)

        # pop schedule: norms of unit k-1 at the unit boundary (their bc
        # matmuls fill the PE while ACT drains the st backlog); outprojs at
        # late jb slots once their o4 region is fully normalized
        outproj_sched = {3: [0, 1], 4: [2, 3], 5: [4, 5, 6], 6: [7, 8, 9], 7: [10, 11]}
        for ui in range(2, 8):
            stt = start_unit(ui)
            if ui == 2:
                pops = [norms[0][0], norms[0][1], norms[1][0], norms[1][1]]
            else:
                pops = [norms[ui - 1][0], norms[ui - 1][1]]
                pops += [make_outproj(tb) for tb in outproj_sched[ui]]
            for cb in pops[0:2]:
                cb()
            extra = pops[2:]
            slots = [12, 14] if len(extra) <= 2 else [10, 12, 14]
            pops_at = {slots[i]: extra[i] for i in range(len(extra))}
            unit_jbs(stt, 0, 16, pops_at)
            finish_unit(stt)

        for cb in norms[7]:
            cb()
        for tb in range(12, 16):
            make_outproj(tb, split_copy=True)()

    nc.compile()
    return nc


def _rpe_factors(w1_rpe, b1_rpe, w2_rpe, b2_rpe):
    """Per head: rank-63 factors of the smoothed bias + exp band buffer.

    Row RANK_SVD (=63) of gmov is all-ones; the matching fstat row is filled
    per-core with the b_qkv correction (zero when b_qkv == 0)."""
    d = np.arange(-(T - 1), T, dtype=np.float64)
    rel = np.sign(d) * np.log1p(np.abs(d))
    hmid = np.maximum(rel[:, None] * np.asarray(w1_rpe, np.float64)[0][None, :]
                      + np.asarray(b1_rpe, np.float64)[None, :], 0.0)
    table = hmid @ np.asarray(w2_rpe, np.float64) + np.asarray(b2_rpe, np.float64)  # [4095, H]

    i0 = (T - 1) - KW
    i1 = (T - 1) + KW
    span = i1 - i0
    tt = (np.arange(i0, i1 + 1) - i0) / span
    h00 = 2 * tt**3 - 3 * tt**2 + 1
    h10 = tt**3 - 2 * tt**2 + tt
    h01 = -2 * tt**3 + 3 * tt**2
    h11 = tt**3 - tt**2
    ii = np.arange(T)
    Dm = ii[:, None] - ii[None, :]
    rng = np.random.default_rng(0)

    gmov = np.zeros((H, RANK, T), np.float16)
    fstat = np.zeros((H, RANK, T), np.float16)
    wbb = np.empty((H, 128, WBB), np.float16)
    for h in range(H):
        tb = table[:, h].copy()
        s0, s1 = tb[i0], tb[i1]
        g0, g1 = tb[i0] - tb[i0 - 1], tb[i1 + 1] - tb[i1]
        tbs = tb.copy()
        tbs[i0 : i1 + 1] = h00 * s0 + h10 * g0 * span + h01 * s1 + h11 * g1 * span
        band = tb - tbs  # support |d| <= KW

        Bs = tbs[Dm + T - 1]
        Y = Bs @ rng.standard_normal((T, RANK_SVD + 33))
        for _ in range(2):
            Y = Bs @ (Bs.T @ Y)
        Q, _ = np.linalg.qr(Y)
        Uc, S, Vt = np.linalg.svd(Q.T @ Bs, full_matrices=False)
        U = Q @ Uc
        gmov[h, :RANK_SVD] = (U[:, :RANK_SVD] * np.sqrt(S[:RANK_SVD])).T.astype(np.float16)
        fstat[h, :RANK_SVD] = (Vt[:RANK_SVD].T * np.sqrt(S[:RANK_SVD])).T.astype(np.float16)
        gmov[h, RANK_SVD] = 1.0  # bias-correction row (moving/query side)

        eb = np.ones(1535, np.float64)
        dd = np.arange(-KW, KW + 1)
        eb[dd + 766] = np.exp(band[dd + T - 1])
        sw = np.lib.stride_tricks.sliding_window_view(eb.astype(np.float16), WBB)
        wbb[h] = sw[::-1]  # wbb[h][p, x] = eb(x - p - 639)
    return gmov, fstat, wbb


def make_core_inputs(x, w_qkv, b_qkv, w_out, b_out, w1_rpe, b1_rpe, w2_rpe, b2_rpe):
    """Build the 8 per-core input dicts from the full problem inputs."""
    x = np.asarray(x, dtype=np.float32)
    w_qkv = np.asarray(w_qkv, dtype=np.float32)
    b_qkv = np.asarray(b_qkv, dtype=np.float32)
    w_out = np.asarray(w_out, dtype=np.float32)

    gmov, fstat, wbb = _rpe_factors(w1_rpe, b1_rpe, w2_rpe, b2_rpe)

    scale = np.float32(1.0 / np.sqrt(DH))
    # b_qkv correction rows: scores softmax drops per-query constants, so only
    # bq·k survives; it is linear in x_j -> fstat row RANK_SVD per (batch, head).
    Wk = w_qkv[:, C : 2 * C]
    bq_full = b_qkv[:C]
    bk_full = b_qkv[C : 2 * C]
    rvec = np.zeros((B, H, T), np.float64)
    if np.any(bq_full):
        for h in range(H):
            bq_h = bq_full[h * DH : (h + 1) * DH]
            u_h = Wk[:, h * DH : (h + 1) * DH] @ bq_h
            cst = float(bq_h @ bk_full[h * DH : (h + 1) * DH])
            for b in range(B):
                rvec[b, h] = (x[b] @ u_h + cst) * scale

    in_maps = []
    for c in range(N_CORES):
        bc = c // 4
        hb = HEADS_PER_CORE * (c % 4)

        xt = np.ascontiguousarray(x[bc].T).reshape(NKC, 128, T)

        # consumer sweeps: 0=Q heads(0,1), 1=K heads(0,1), 2=Q heads(2,3), 3=K heads(2,3)
        wqk = np.empty((128, HEADS_PER_CORE * NKC * 128), np.float32)
        for cons in range(4):
            a, is_k = cons // 2, cons % 2
            h0g, h1g = hb + 2 * a, hb + 2 * a + 1
            off = C if is_k else 0
            sc = np.float32(1.0) if is_k else scale
            blk = np.concatenate(
                [w_qkv[:, off + h0g * DH : off + (h0g + 1) * DH] * sc,
                 w_qkv[:, off + h1g * DH : off + (h1g + 1) * DH] * sc], axis=1)
            for kc in range(NKC):
                wqk[:, (cons * NKC + kc) * 128 : (cons * NKC + kc + 1) * 128] = blk[kc * 128 : (kc + 1) * 128]

        # wv: [c, d] blocks -> [128, NKC * 4*DH] (kc-major along free dim)
        wv_cd = w_qkv[:, 2 * C + hb * DH : 2 * C + (hb + 4) * DH]  # [C, 256]
        wv = np.ascontiguousarray(
            wv_cd.reshape(NKC, 128, 4 * DH).transpose(1, 0, 2).reshape(128, NKC * 4 * DH))

        wout = np.empty((128, 2 * C), np.float32)
        for a in range(2):
            wout[:, a * C : (a + 1) * C] = w_out[hb * DH + a * 128 : hb * DH + (a + 1) * 128, :]

        fst = np.ascontiguousarray(fstat[hb : hb + HEADS_PER_CORE])
        for hi in range(HEADS_PER_CORE):
            fst[hi, RANK_SVD] = rvec[bc, hb + hi].astype(np.float16)

        in_maps.append({
            "xt": np.ascontiguousarray(xt).astype(ml_dtypes.bfloat16),
            "wqk": wqk.astype(ml_dtypes.bfloat16),
            "wv": wv.astype(ml_dtypes.bfloat16),
            "wout": wout.astype(np.float16),
            "gmov": np.ascontiguousarray(gmov[hb : hb + HEADS_PER_CORE].transpose(1, 0, 2)),
            "fstat": np.ascontiguousarray(fst.transpose(1, 0, 2)),
            "wbb": np.ascontiguousarray(wbb[hb : hb + HEADS_PER_CORE].transpose(1, 0, 2)),
            "onesr": np.ones((1, DH), np.float16),
        })
    return in_maps


_PROGRAM = None


def kernel(x, w_qkv, b_qkv, w_out, b_out, w1_rpe, b1_rpe, w2_rpe, b2_rpe):
    global _PROGRAM
    if _PROGRAM is None:
        _PROGRAM = build_program()
    nc = _PROGRAM

    in_maps = make_core_inputs(x, w_qkv, b_qkv, w_out, b_out, w1_rpe, b1_rpe, w2_rpe, b2_rpe)
    res = run_bass_kernel_spmd(nc, in_maps, core_ids=list(range(N_CORES)), trace=False)

    return assemble(res.results, b_out, w_qkv=w_qkv, b_qkv=b_qkv, w_out=w_out)


def assemble(results, b_out, w_qkv=None, b_qkv=None, w_out=None):
    b_out = np.asarray(b_out, dtype=np.float32).copy()
    if b_qkv is not None:
        bv_full = np.asarray(b_qkv, np.float32)[2 * C :]
        if np.any(bv_full):
            b_out = b_out + bv_full @ np.asarray(w_out, np.float32)
    out = np.empty((B, T, C), np.float32)
    for b in range(B):
        acc = results[4 * b]["out"].astype(np.float32)
        for c in range(4 * b + 1, 4 * b + 4):
            acc = acc + results[c]["out"].astype(np.float32)
        out[b] = acc + b_out[None, :]
    return out
